# revision 22
# baseline (speedup 1.0000x reference)
"""Trainium2 Bass kernel for nn_BlockShuffleLayer (butterfly block-diag MLP).

Math (reference):
  out1[b, k, q] = sum_p x[b, k*256+p] * w1[k, q, p]          (k=16 blocks, p=q=256)
  shuffle: kq index (k*256+q) viewed as (r, l), r=kq//16, l=kq%16
  out2[b, s, l] = sum_r out1s[b, l, r] * w2[l, s, r]          (l=16 blocks, r=256, s=1024)
  out[b, s*16+l] = out2[b, s, l]

Strategy: data-parallel over the 4096-token batch across 8 cores (512 tokens
each), weights replicated.  The whole pipeline runs in bf16 (fp32 PSUM
accumulation): the task tolerance is 2e-2 and end-to-end bf16 lands ~2e-3,
while halving every byte of HBM/DMA traffic vs fp32(r) -- this kernel is
DMA-bound, so bytes are time.  Per core:

  phase A (stage 1, output feature-major):
    - x arrives host-transposed (xt[p, b]) so the contraction dim is already
      on partitions: zero on-chip transposes, and the tensor engine runs a
      pure back-to-back matmul stream (keeps the HAM clock warm).  w1 is
      streamed per-block so the first matmul isn't gated on a bulk load.
    - stage-1 matmuls produce out1T[q'', b] in PSUM (64 x N=512) with w1
      column-permuted on host so the butterfly shuffle becomes 16-partition
      stripe moves; k-blocks are processed in (k, k+8) pairs whose stripes
      share partitions, so one SBUF->SBUF DMA per (pair, stripe) scatters 4
      stripes at once into the z layout with r naturally ordered for w2.
  phase B (stage 2, tokens-major):
    - w2 resident as per-(s-half, l) tiles, double-buffered across halves:
      the sh=1 tiles stream during sh=0 compute with no WAR stall
    - psum[b, s] scatter-copied (stride-16 SBUF writes, DVE/ACT alternating)
      into the interleaved output columns, then contiguous 16KB/partition
      DMAs out.

Queue discipline: the two HWDGE rings (sync, scalar) carry PURE LOAD
streams (w1+x on sync, w2 on scalar) so they never stall on a data
dependency and prefetch arbitrarily deep; all store-side DMAs (butterfly
scatter, output stores), which must wait on engine copies, sit on the
gpsimd SWDGE queue where their dep-stalls block nothing else.  Deep tile
pools (psum 8 banks, 3-4x on stream buffers) let the Tile scheduler run
loads several k-blocks ahead.

Per-core per-invocation budget (bf16): HBM traffic x 4MB + w1 2MB +
w2 8MB + out 16MB = 30MB (~75-85us at the ~360GB/s/core paper roofline);
PE 320 matmuls x 512 moving cols = 164k cycles (~68us warm) -- a genuine
ridge-regime kernel.  Measured steady-state (3-point reps-slope, all 8
cores streaming concurrently) is ~257us/rep, i.e. ~117GB/s effective per
core -- HBM-stack contention under full-chip load; per-rep time tracks
bytes moved, which are at the algorithmic floor here.
"""

import numpy as np
import ml_dtypes

import concourse.bacc as bacc
import concourse.bass as bass
import concourse.mybir as mybir
import concourse.tile as tile
from concourse import bass_utils

FP32 = mybir.dt.float32
BF16 = mybir.dt.bfloat16
MMDT = BF16
NPDT = ml_dtypes.bfloat16

K, Q, P = 16, 256, 256
L, S, R = 16, 1024, 256
N_IN = K * P          # 4096
N_OUT = S * L         # 16384
BATCH = 4096
NCORES = 8
SHARD = BATCH // NCORES


def build_kernel(n_tokens: int = SHARD, reps: int = 1,
                 serialize_reps: bool = False) -> bass.Bass:
    nbc = n_tokens // 128
    nc = bacc.Bacc("TRN2", target_bir_lowering=False, debug=False,
                   num_devices=NCORES)

    # host-prepared layouts (see _prep_weights / kernel):
    #   xt[P, b]                      = x[b, P]  (pre-transposed shard)
    #   w1t[p, k, pc, qc*128+u]       = w1[k, (u//8)*16 + qc*8 + u%8, pc*128+p]
    #   w2t[sh, r', l, rc, s']        = w2[l, sh*512+s', rc*128+r']
    xt = nc.dram_tensor("xt", [128, 8, 2, 2, n_tokens], MMDT,
                        kind="ExternalInput")
    w1t = nc.dram_tensor("w1t", [128, K, 2, Q], MMDT, kind="ExternalInput")
    w2t = nc.dram_tensor("w2t", [2, 128, L, 2, 512], MMDT,
                         kind="ExternalInput")
    out = nc.dram_tensor("out", [n_tokens, N_OUT], MMDT, kind="ExternalOutput")

    with tile.TileContext(nc) as tc:
        with tc.tile_pool(name="const", bufs=1) as cpool:
            # z[u', l, rc, b]: shuffled stage-1 output; r = rc*128 + u'
            z_sb = cpool.tile([128, L, 2, n_tokens], MMDT)
            # w2 as per-(s-half, l-pair) tiles [r', l%2, rc, s'], double-
            # buffered across halves so the sh=1 loads stream during sh=0
            # compute with no WAR stall
            w2p = [[cpool.tile([128, 2, 2, 512], MMDT, name=f"w2p{sh}_{lp}")
                    for lp in range(L // 2)] for sh in range(2)]

            def phase_a():
                with tc.tile_pool(name="pa", bufs=1) as pa, \
                     tc.tile_pool(name="pap", bufs=8, space="PSUM") as pap:
                    # process k-pairs (k0, k0+8): their stripes land in the
                    # same 16 z partitions (differing only in the rc slot),
                    # so one DMA per (pair, t) scatters 4 stripes at once
                    for k0 in range(8):
                        # prefetch first w2 half spread behind stage-1
                        # compute; per-DMA fixed cost dominates this
                        # environment, so load l-pairs in one DMA each
                        nc.scalar.dma_start(
                            w2p[0][k0][:], w2t[0, :, 2 * k0:2 * k0 + 2])
                        stg = pa.tile([128, 2, 2, n_tokens], MMDT, tag="stg",
                                      name="stg", bufs=3)   # [u, qc, kh, b]
                        # one DMA each for both kh halves of w1 / x
                        w1k = pa.tile([128, 2, 2, Q], MMDT, tag="w1k",
                                      name="w1k", bufs=4)   # [p, kh, pc, q]
                        nc.sync.dma_start(w1k[:], w1t[:, k0:k0 + 9:8])
                        xtk = pa.tile([128, 2, 2, n_tokens], MMDT, tag="xtk",
                                      name="xtk", bufs=4)   # [p, kh, pc, b]
                        nc.sync.dma_start(xtk[:], xt[:, k0])
                        for kh in range(2):
                            for qc in range(2):
                                ps1 = pap.tile([128, n_tokens], FP32,
                                               tag="ps1", name="ps1")
                                for pc in range(2):
                                    nc.tensor.matmul(
                                        ps1[:],
                                        w1k[:, kh, pc,
                                            qc * 128:(qc + 1) * 128],
                                        xtk[:, kh, pc, :],
                                        start=(pc == 0), stop=(pc == 1))
                                if (kh + qc) % 2 == 0:
                                    nc.vector.tensor_copy(
                                        stg[:, qc, kh, :], ps1[:])
                                else:
                                    nc.scalar.copy(stg[:, qc, kh, :], ps1[:])
                        # butterfly redistribution: psum partition u = 16t+j
                        # holds column (l = qc*8+t, j); z row u' = k0*16+j,
                        # rc = kh, so r = rc*128+u' is natural for w2.  On
                        # the HWDGE rings (~0.6us fixed vs ~2us SWDGE).
                        for t in range(8):
                            eng = nc.sync if t % 2 == 0 else nc.scalar
                            eng.dma_start(
                                z_sb[k0 * 16:k0 * 16 + 16, t:t + 9:8, :, :],
                                stg[16 * t:16 * t + 16, :, :, :])

            def phase_b():
                with tc.tile_pool(name="pb", bufs=4) as pb, \
                     tc.tile_pool(name="pbp", bufs=8, space="PSUM") as pbp:
                    for sh in range(2):
                        for bc in range(nbc):
                            if sh == 0:
                                # stream the sh=1 w2 tiles (independent
                                # buffers) behind the sh=0 compute
                                for lp in (2 * bc, 2 * bc + 1):
                                    nc.scalar.dma_start(
                                        w2p[1][lp][:],
                                        w2t[1, :, 2 * lp:2 * lp + 2])
                            ob = pb.tile([128, 512 * L], MMDT, tag="ob",
                                         name="ob")
                            ob3 = ob[:].rearrange("p (s l) -> p s l", l=L)
                            for l in range(L):
                                ps2 = pbp.tile([128, 512], FP32, tag="ps2",
                                               name="ps2")
                                for rc in range(2):
                                    nc.tensor.matmul(
                                        ps2[:],
                                        z_sb[:, l, rc, bc * 128:(bc + 1) * 128],
                                        w2p[sh][l // 2][:, l % 2, rc, :],
                                        start=(rc == 0), stop=(rc == 1))
                                if l % 2 == 0:
                                    nc.vector.tensor_copy(ob3[:, :, l], ps2[:])
                                else:
                                    nc.scalar.copy(ob3[:, :, l], ps2[:])
                            nc.gpsimd.dma_start(
                                out[bc * 128:(bc + 1) * 128,
                                    sh * 8192:(sh + 1) * 8192],
                                ob[:])

            for _rep in range(reps):
                phase_a()
                phase_b()
                if serialize_reps and _rep != reps - 1:
                    # benchmarking only: forbid cross-rep overlap so the
                    # reps-slope measures a full single-invocation span
                    tc.strict_bb_all_engine_barrier()

    nc.compile()
    return nc


# stage-1 psum chunk qc, partition u = 16t+j holds output column
# q = j*16 + (qc*8 + t)
_QCOL = np.array([(u % 16) * 16 + (qc * 8) + u // 16
                  for qc in range(2) for u in range(128)])


def _prep_weights(w1: np.ndarray, w2: np.ndarray):
    # w1t[p, k, pc, q''] = w1[k, _QCOL[q''], pc*128+p]
    w1p = w1.astype(NPDT)[:, _QCOL, :]           # [k, q'', P]
    w1t = np.ascontiguousarray(
        w1p.reshape(K, Q, 2, 128).transpose(3, 0, 2, 1))
    # w2t[sh, r', l, rc, s'] = w2[l, sh*512+s', rc*128+r']
    w2t = np.ascontiguousarray(
        w2.astype(NPDT).reshape(L, 2, 512, 2, 128).transpose(1, 4, 0, 3, 2))
    return w1t, w2t


def _prep_x(x: np.ndarray):
    # xt3[i][p, k0, kh, pc, b] = x[i*SHARD+b, (k0 + 8*kh)*256 + pc*128 + p]
    # so each (k0, k0+8) block pair is one contiguous dram slice per core
    xb = x.astype(NPDT)
    return np.ascontiguousarray(
        xb.reshape(NCORES, SHARD, 2, 8, 2, 128).transpose(0, 5, 3, 2, 4, 1))


_NC_CACHE: dict = {}


def kernel(x, w1, w2) -> np.ndarray:
    x = np.asarray(x, dtype=np.float32)
    w1 = np.asarray(w1, dtype=np.float32)
    w2 = np.asarray(w2, dtype=np.float32)
    assert x.shape == (BATCH, N_IN) and w1.shape == (K, Q, P) \
        and w2.shape == (L, S, R)

    if "nc" not in _NC_CACHE:
        _NC_CACHE["nc"] = build_kernel(SHARD)
    nc = _NC_CACHE["nc"]

    w1t, w2t = _prep_weights(w1, w2)
    xt3 = _prep_x(x)
    in_maps = [{"xt": xt3[i], "w1t": w1t, "w2t": w2t} for i in range(NCORES)]
    res = bass_utils.run_bass_kernel_spmd(nc, in_maps,
                                          core_ids=list(range(NCORES)))
    outs = [r["out"] for r in res.results]
    # exact bf16 -> fp32 upconversion (bit shift), then concat shards
    full = np.empty((BATCH, N_OUT), np.float32)
    for i, o in enumerate(outs):
        u = np.asarray(o).view(np.uint16).astype(np.uint32) << 16
        full[i * SHARD:(i + 1) * SHARD] = u.view(np.float32)
    return full
